# revision 24
# baseline (speedup 1.0000x reference)
"""Bahdanau attention forward on 8 Trainium2 NeuronCores (Bass/Tile).

Data-parallel: batch 32 sharded 4-per-core; weights replicated. Per core:
  enc_projT[d, s] = sum_e W_enc[e, d] * X[b, s, e]        (bf16 matmul, fp32 acc)
  energyT = tanh(enc_projT + dec_projT[d, b])             (ACT, per-partition bias)
  scores[s] = sum_d v[d] * energyT[d, s]                  (bf16 matmul)
  weights = softmax(scores + maskneg)                     (DVE/ACT, partition 0)
  context[e] = sum_s weights[s] * X[b, s, e]              (bf16 matmul)

X is shipped pre-cast to bf16 (host-side dtype prep; all FLOPs stay on
device). X^T chunks are produced by x-bar transpose DMAs *directly from
DRAM*, alternating between the two HWDGE queues (each transpose moves in
256B packets at ~150 GB/s, so parallelism + deep prefetch matter); the
natural copy used by the context matmul is one contiguous load per batch.
The score matmul for d-tile i issues after d-tile i+1's enc matmuls so PE
never waits on the tanh ACT; context matmuls of batch b are emitted under
batch b+1's first chunk so the softmax chain hides under enc matmuls.
"""

from contextlib import ExitStack

import numpy as np
import ml_dtypes

import concourse.bass as bass  # noqa: F401
import concourse.mybir as mybir
import concourse.tile as tile
from concourse import bacc
from concourse.bass_utils import run_bass_kernel_spmd

F32 = mybir.dt.float32
BF16 = mybir.dt.bfloat16
AFT = mybir.ActivationFunctionType

B, S, E, D = 32, 2048, 1024, 1024
N_CORES = 8
BL = B // N_CORES


def build_program(BL=4, S=2048, E=1024, D=1024, schunk=512):
    """Build and compile the per-core SPMD program. Returns nc."""
    P = 128
    ST = S // P               # s tiles
    STC = schunk // P         # s tiles per chunk
    NCH = ST // STC           # chunks per batch
    ET = E // P
    DT = D // P
    ECH = 512                 # context matmul free-dim chunk
    EC = (E + ECH - 1) // ECH

    nc = bacc.Bacc("TRN2", target_bir_lowering=False, debug=False)

    Xd = nc.declare_dram_parameter("x", [BL, S, E], BF16, isOutput=False)
    HTd = nc.declare_dram_parameter("ht", [E, BL], BF16, isOutput=False)
    MNd = nc.declare_dram_parameter("maskneg", [BL, S], F32, isOutput=False)
    # host-prepped SBUF-layouts (contiguous partition-major loads)
    WEd = nc.declare_dram_parameter("wenc", [P, DT, ET, P], BF16, isOutput=False)
    WDd = nc.declare_dram_parameter("wdec", [P, DT, ET, P], BF16, isOutput=False)
    VTd = nc.declare_dram_parameter("vt", [P, DT], BF16, isOutput=False)
    CTXd = nc.declare_dram_parameter("context", [BL, E], F32, isOutput=True)
    WTSd = nc.declare_dram_parameter("weights", [BL, S], F32, isOutput=True)

    with tile.TileContext(nc) as tc, ExitStack() as ctx:
        consts = ctx.enter_context(tc.tile_pool(name="consts", bufs=1))
        xn_pool = ctx.enter_context(tc.tile_pool(name="xn", bufs=2))
        xt_pool = ctx.enter_context(tc.tile_pool(name="xt", bufs=4))
        work = ctx.enter_context(tc.tile_pool(name="work", bufs=4))
        sm_pool = ctx.enter_context(tc.tile_pool(name="sm", bufs=2))
        dram_pool = ctx.enter_context(tc.tile_pool(name="dram", bufs=2, space="DRAM"))
        ps_e_pool = ctx.enter_context(tc.tile_pool(name="pse", bufs=2, space="PSUM"))
        ps_s_pool = ctx.enter_context(tc.tile_pool(name="pss", bufs=2, space="PSUM"))
        ps_c_pool = ctx.enter_context(tc.tile_pool(name="psc", bufs=1, space="PSUM"))
        ps_d_pool = ctx.enter_context(tc.tile_pool(name="psd", bufs=1, space="PSUM"))

        # transposed chunk loads directly from DRAM, alternating HWDGE queues
        def emit_xt(b, ci, off, w):
            xt_sb = xt_pool.tile([P, ET, w * P], BF16, tag="xt")
            eng = nc.sync if (ci % 2 == 0) else nc.scalar
            eng.dma_start(
                out=xt_sb,
                in_=Xd.ap()[b, off * P : (off + w) * P, :],
                transpose=True,
            )
            return xt_sb

        # ---- prologue emission ----
        # scalar q: W_enc, then batch-0 odd transposes; sync q: W_dec, then
        # batch-0 even transposes. SWDGE: small consts.
        wenc_sb = consts.tile([P, DT, ET, P], BF16)
        nc.scalar.dma_start(out=wenc_sb[:, 0, :, :], in_=WEd.ap()[:, 0, :, :])
        nc.scalar.dma_start(out=wenc_sb[:, 1:, :, :], in_=WEd.ap()[:, 1:, :, :])
        wdall_sb = consts.tile([P, DT, ET, P], BF16)
        nc.sync.dma_start(out=wdall_sb, in_=WDd.ap())
        vt_sb = consts.tile([P, DT], BF16)
        nc.gpsimd.dma_start(out=vt_sb, in_=VTd.ap())
        ht_sb = consts.tile([P, ET, BL], BF16)
        nc.gpsimd.dma_start(
            out=ht_sb, in_=HTd.ap().rearrange("(et p) b -> p et b", p=P)
        )

        # dec_projT[d, b] = sum_e W_dec[e, d] h[b, e]  (PE + DVE copies)
        decT_sb = consts.tile([P, DT, BL], F32)
        for dt_i in range(DT):
            ps_d = ps_d_pool.tile([P, BL], F32, tag="psd")
            for et in range(ET):
                nc.tensor.matmul(
                    ps_d,
                    lhsT=wdall_sb[:, dt_i, et, :],
                    rhs=ht_sb[:, et, :],
                    start=(et == 0),
                    stop=(et == ET - 1),
                )
            nc.vector.tensor_copy(out=decT_sb[:, dt_i, :], in_=ps_d)

        def emit_context(prev):
            """Context matmuls + store for a finished batch."""
            pb, xn_sb, wt_sb, rinv = prev
            ps_c = ps_c_pool.tile([1, E], F32, tag="psc")
            for st in range(ST):
                for ec2 in range(EC):
                    nc.tensor.matmul(
                        ps_c[:, ec2 * ECH : (ec2 + 1) * ECH],
                        lhsT=wt_sb[:, st : st + 1],
                        rhs=xn_sb[:, st, ec2 * ECH : (ec2 + 1) * ECH],
                        start=(st == 0),
                        stop=(st == ST - 1),
                    )
            ctx_sb = sm_pool.tile([1, E], F32, tag="ctx", bufs=1)
            nc.vector.tensor_scalar_mul(ctx_sb, ps_c, rinv)
            nc.gpsimd.dma_start(out=CTXd.ap()[pb : pb + 1, :], in_=ctx_sb)

        # ---- per-batch pipeline ----
        prev = None
        for b in range(BL):
            mn_sb = sm_pool.tile([1, S], F32, tag="mn")
            nc.gpsimd.dma_start(out=mn_sb, in_=MNd.ap()[b : b + 1, :])
            scores_sb = sm_pool.tile([1, S], F32, tag="scores")
            xn_sb = None

            for ci in range(NCH):
                off = ci * STC
                w = STC
                width = w * P
                xt_sb = emit_xt(b, ci, off, w)

                ps_s = ps_s_pool.tile([1, width], F32, tag="pss")
                ens = []
                for dt_i in range(DT):
                    ps_e = ps_e_pool.tile([P, width], F32, tag="pse")
                    for et in range(ET):
                        nc.tensor.matmul(
                            ps_e,
                            lhsT=wenc_sb[:, dt_i, et, :],
                            rhs=xt_sb[:, et, :],
                            start=(et == 0),
                            stop=(et == ET - 1),
                        )
                    # score matmul lags two d-tiles so PE never waits on ACT
                    if dt_i >= 2:
                        nc.tensor.matmul(
                            ps_s,
                            lhsT=vt_sb[:, dt_i - 2 : dt_i - 1],
                            rhs=ens[dt_i - 2],
                            start=(dt_i - 2 == 0),
                            stop=False,
                        )
                    en_sb = work.tile([P, width], BF16, tag="energy")
                    nc.scalar.activation(
                        out=en_sb,
                        in_=ps_e,
                        func=AFT.Tanh,
                        bias=decT_sb[:, dt_i, b : b + 1],
                    )
                    ens.append(en_sb)
                for dt_i in (DT - 2, DT - 1):
                    nc.tensor.matmul(
                        ps_s,
                        lhsT=vt_sb[:, dt_i : dt_i + 1],
                        rhs=ens[dt_i],
                        start=False,
                        stop=(dt_i == DT - 1),
                    )
                # psum -> scores row (with additive mask fold-in)
                nc.vector.tensor_add(
                    out=scores_sb[:, off * P : off * P + width],
                    in0=ps_s,
                    in1=mn_sb[:, off * P : off * P + width],
                )
                # previous batch's context hides under this batch's first chunk
                if ci == 0 and prev is not None:
                    emit_context(prev)
                    prev = None
                # natural copy for the context matmul (one contiguous load,
                # emitted after the first transposes so it doesn't block them)
                if ci == min(1, NCH - 1):
                    xn_sb = xn_pool.tile([P, ST, E], BF16, tag="xn")
                    nc.scalar.dma_start(
                        out=xn_sb,
                        in_=Xd.ap()[b].rearrange("(st p) e -> p st e", p=P),
                    )

            # softmax over s on partition 0 (in-place on scores_sb)
            negmax = sm_pool.tile([1, 1], F32, tag="negmax")
            nc.vector.tensor_reduce(
                out=negmax,
                in_=scores_sb,
                axis=mybir.AxisListType.X,
                op=mybir.AluOpType.max,
                negate=True,
            )
            ssum = sm_pool.tile([1, 1], F32, tag="ssum")
            nc.scalar.activation(
                out=scores_sb,
                in_=scores_sb,
                func=AFT.Exp,
                bias=negmax,
                accum_out=ssum,
            )
            rinv = sm_pool.tile([1, 1], F32, tag="rinv")
            nc.vector.reciprocal(out=rinv, in_=ssum)

            # w^T via DRAM bounce (unnormalized exp row — context scales by
            # rinv): flat write, strided cast read to [sp, st]
            wq_dram = dram_pool.tile([S], F32, tag="wq")
            nc.gpsimd.dma_start(out=wq_dram, in_=scores_sb)
            wt_sb = sm_pool.tile([P, ST], BF16, tag="wt")
            nc.gpsimd.dma_start(
                out=wt_sb, in_=wq_dram[:].rearrange("(st sp) -> sp st", sp=P)
            )
            # normalize -> weights output row
            nc.vector.tensor_scalar_mul(scores_sb, scores_sb, rinv)
            nc.gpsimd.dma_start(out=WTSd.ap()[b : b + 1, :], in_=scores_sb)
            prev = (b, xn_sb, wt_sb, rinv)

        emit_context(prev)

    nc.compile()
    return nc


def host_inputs(decoder_hidden, encoder_outputs, mask, W_enc, W_dec, v,
                n_cores=N_CORES):
    """Shard + prep host-side numpy inputs; returns per-core input maps."""
    nb, Dd = decoder_hidden.shape
    bl = nb // n_cores
    Ee = W_enc.shape[0]
    ET, DT = Ee // 128, Dd // 128
    maskneg = np.where(mask, np.float32(-1e30), np.float32(0.0)).astype(np.float32)
    xb = encoder_outputs.astype(ml_dtypes.bfloat16)
    # W_enc in dt-major SBUF layout [p, dt, et, c]
    wenc_b = np.ascontiguousarray(
        W_enc.astype(ml_dtypes.bfloat16)
        .reshape(ET, 128, DT, 128)
        .transpose(1, 2, 0, 3)
    )
    # W_dec in strip-major SBUF layout [dt, p, et, c]
    wdec_b = np.ascontiguousarray(
        W_dec.astype(ml_dtypes.bfloat16)
        .reshape(ET, 128, DT, 128)
        .transpose(1, 2, 0, 3)
    )
    vt = np.ascontiguousarray(v.astype(ml_dtypes.bfloat16).reshape(DT, 128).T)
    hT = np.ascontiguousarray(decoder_hidden.T).astype(ml_dtypes.bfloat16)
    in_maps = []
    for c in range(n_cores):
        sl = slice(c * bl, (c + 1) * bl)
        in_maps.append(
            {
                "x": np.ascontiguousarray(xb[sl]),
                "ht": np.ascontiguousarray(hT[:, sl]),
                "maskneg": np.ascontiguousarray(maskneg[sl]),
                "wenc": wenc_b,
                "wdec": wdec_b,
                "vt": vt,
            }
        )
    return in_maps


_CACHE = {}


def _get_program():
    if "nc" not in _CACHE:
        _CACHE["nc"] = build_program(BL=BL, S=S, E=E, D=D)
    return _CACHE["nc"]


def run(inputs, trace=False):
    """inputs: dict as from setup_inputs(); returns (results, context, weights)."""
    nc = _get_program()
    in_maps = host_inputs(
        np.asarray(inputs["decoder_hidden"], dtype=np.float32),
        np.asarray(inputs["encoder_outputs"], dtype=np.float32),
        np.asarray(inputs["mask"]),
        np.asarray(inputs["W_enc"], dtype=np.float32),
        np.asarray(inputs["W_dec"], dtype=np.float32),
        np.asarray(inputs["v"], dtype=np.float32),
    )
    res = run_bass_kernel_spmd(nc, in_maps, list(range(N_CORES)), trace=trace)
    context = np.concatenate(
        [np.asarray(r["context"], dtype=np.float32) for r in res.results], axis=0
    )
    weights = np.concatenate(
        [np.asarray(r["weights"], dtype=np.float32) for r in res.results], axis=0
    )
    return res, context, weights


def kernel(decoder_hidden, encoder_outputs, mask, W_enc, W_dec, v):
    _, context, weights = run(
        {
            "decoder_hidden": decoder_hidden,
            "encoder_outputs": encoder_outputs,
            "mask": mask,
            "W_enc": W_enc,
            "W_dec": W_dec,
            "v": v,
        }
    )
    return context, weights


# revision 25
# speedup vs baseline: 1.0227x; 1.0227x over previous
"""Bahdanau attention forward on 8 Trainium2 NeuronCores (Bass/Tile).

Data-parallel: batch 32 sharded 4-per-core; weights replicated. Per core:
  enc_projT[d, s] = sum_e W_enc[e, d] * X[b, s, e]        (bf16 matmul, fp32 acc)
  energyT = tanh(enc_projT + dec_projT[d, b])             (ACT, per-partition bias)
  scores[s] = sum_d v[d] * energyT[d, s]                  (bf16 matmul)
  weights = softmax(scores + maskneg)                     (DVE/ACT, partition 0)
  context[e] = sum_s weights[s] * X[b, s, e]              (bf16 matmul)

X is shipped pre-cast to bf16 (host-side dtype prep; all FLOPs stay on
device). X^T chunks are produced by x-bar transpose DMAs *directly from
DRAM*, alternating between the two HWDGE queues (each transpose moves in
256B packets at ~150 GB/s, so parallelism + deep prefetch matter); the
natural copy used by the context matmul is one contiguous load per batch.
The score matmul for d-tile i issues after d-tile i+1's enc matmuls so PE
never waits on the tanh ACT; context matmuls of batch b are emitted under
batch b+1's first chunk so the softmax chain hides under enc matmuls.
"""

from contextlib import ExitStack

import numpy as np
import ml_dtypes

import concourse.bass as bass  # noqa: F401
import concourse.mybir as mybir
import concourse.tile as tile
from concourse import bacc
from concourse.bass_utils import run_bass_kernel_spmd

F32 = mybir.dt.float32
BF16 = mybir.dt.bfloat16
AFT = mybir.ActivationFunctionType

B, S, E, D = 32, 2048, 1024, 1024
N_CORES = 8
BL = B // N_CORES


def build_program(BL=4, S=2048, E=1024, D=1024, schunk=512):
    """Build and compile the per-core SPMD program. Returns nc."""
    P = 128
    ST = S // P               # s tiles
    STC = schunk // P         # s tiles per chunk
    NCH = ST // STC           # chunks per batch
    ET = E // P
    DT = D // P
    ECH = 512                 # context matmul free-dim chunk
    EC = (E + ECH - 1) // ECH

    nc = bacc.Bacc("TRN2", target_bir_lowering=False, debug=False)

    Xd = nc.declare_dram_parameter("x", [BL, S, E], BF16, isOutput=False)
    HTd = nc.declare_dram_parameter("ht", [E, BL], BF16, isOutput=False)
    MNd = nc.declare_dram_parameter("maskneg", [BL, S], F32, isOutput=False)
    # host-prepped SBUF-layouts (contiguous partition-major loads)
    WEd = nc.declare_dram_parameter("wenc", [P, DT, ET, P], BF16, isOutput=False)
    WDd = nc.declare_dram_parameter("wdec", [P, DT, ET, P], BF16, isOutput=False)
    VTd = nc.declare_dram_parameter("vt", [P, DT], BF16, isOutput=False)
    CTXd = nc.declare_dram_parameter("context", [BL, E], F32, isOutput=True)
    WTSd = nc.declare_dram_parameter("weights", [BL, S], F32, isOutput=True)

    with tile.TileContext(nc) as tc, ExitStack() as ctx:
        consts = ctx.enter_context(tc.tile_pool(name="consts", bufs=1))
        xn_pool = ctx.enter_context(tc.tile_pool(name="xn", bufs=2))
        xt_pool = ctx.enter_context(tc.tile_pool(name="xt", bufs=4))
        work = ctx.enter_context(tc.tile_pool(name="work", bufs=4))
        sm_pool = ctx.enter_context(tc.tile_pool(name="sm", bufs=2))
        dram_pool = ctx.enter_context(tc.tile_pool(name="dram", bufs=2, space="DRAM"))
        ps_e_pool = ctx.enter_context(tc.tile_pool(name="pse", bufs=2, space="PSUM"))
        ps_s_pool = ctx.enter_context(tc.tile_pool(name="pss", bufs=2, space="PSUM"))
        ps_c_pool = ctx.enter_context(tc.tile_pool(name="psc", bufs=1, space="PSUM"))
        ps_d_pool = ctx.enter_context(tc.tile_pool(name="psd", bufs=1, space="PSUM"))

        # transposed chunk loads directly from DRAM, alternating HWDGE queues
        def emit_xt(b, ci, off, w):
            xt_sb = xt_pool.tile([P, ET, w * P], BF16, tag="xt")
            eng = nc.sync if (ci % 2 == 0) else nc.scalar
            eng.dma_start(
                out=xt_sb,
                in_=Xd.ap()[b, off * P : (off + w) * P, :],
                transpose=True,
            )
            return xt_sb

        # ---- prologue emission ----
        # scalar q: W_enc, then batch-0 odd transposes; sync q: W_dec, then
        # batch-0 even transposes. SWDGE: small consts.
        wenc_sb = consts.tile([P, DT, ET, P], BF16)
        nc.scalar.dma_start(out=wenc_sb[:, 0, :, :], in_=WEd.ap()[:, 0, :, :])
        nc.scalar.dma_start(out=wenc_sb[:, 1:, :, :], in_=WEd.ap()[:, 1:, :, :])
        wdall_sb = consts.tile([P, DT, ET, P], BF16)
        nc.sync.dma_start(out=wdall_sb, in_=WDd.ap())
        vt_sb = consts.tile([P, DT], BF16)
        nc.gpsimd.dma_start(out=vt_sb, in_=VTd.ap())
        ht_sb = consts.tile([P, ET, BL], BF16)
        nc.gpsimd.dma_start(
            out=ht_sb, in_=HTd.ap().rearrange("(et p) b -> p et b", p=P)
        )

        # dec_projT[d, b] = sum_e W_dec[e, d] h[b, e]  (PE + DVE copies)
        decT_sb = consts.tile([P, DT, BL], F32)
        for dt_i in range(DT):
            ps_d = ps_d_pool.tile([P, BL], F32, tag="psd")
            for et in range(ET):
                nc.tensor.matmul(
                    ps_d,
                    lhsT=wdall_sb[:, dt_i, et, :],
                    rhs=ht_sb[:, et, :],
                    start=(et == 0),
                    stop=(et == ET - 1),
                )
            nc.vector.tensor_copy(out=decT_sb[:, dt_i, :], in_=ps_d)

        def emit_context(prev):
            """Context matmuls + store for a finished batch."""
            pb, xn_sb, wt_sb, rinv = prev
            ps_c = ps_c_pool.tile([1, E], F32, tag="psc")
            for st in range(ST):
                for ec2 in range(EC):
                    nc.tensor.matmul(
                        ps_c[:, ec2 * ECH : (ec2 + 1) * ECH],
                        lhsT=wt_sb[:, st : st + 1],
                        rhs=xn_sb[:, st, ec2 * ECH : (ec2 + 1) * ECH],
                        start=(st == 0),
                        stop=(st == ST - 1),
                    )
            ctx_sb = sm_pool.tile([1, E], F32, tag="ctx", bufs=1)
            nc.vector.tensor_scalar_mul(ctx_sb, ps_c, rinv)
            nc.gpsimd.dma_start(out=CTXd.ap()[pb : pb + 1, :], in_=ctx_sb)

        # ---- per-batch pipeline ----
        prev = None
        for b in range(BL):
            mn_sb = sm_pool.tile([1, S], F32, tag="mn")
            nc.gpsimd.dma_start(out=mn_sb, in_=MNd.ap()[b : b + 1, :])
            scores_sb = sm_pool.tile([1, S], F32, tag="scores")
            xn_sb = None

            for ci in range(NCH):
                off = ci * STC
                w = STC
                width = w * P
                xt_sb = emit_xt(b, ci, off, w)

                ps_s = ps_s_pool.tile([1, width], F32, tag="pss")
                ens = []
                for dt_i in range(DT):
                    ps_e = ps_e_pool.tile([P, width], F32, tag="pse")
                    for et in range(ET):
                        nc.tensor.matmul(
                            ps_e,
                            lhsT=wenc_sb[:, dt_i, et, :],
                            rhs=xt_sb[:, et, :],
                            start=(et == 0),
                            stop=(et == ET - 1),
                        )
                    # score matmul lags two d-tiles so PE never waits on ACT
                    if dt_i >= 2:
                        nc.tensor.matmul(
                            ps_s,
                            lhsT=vt_sb[:, dt_i - 2 : dt_i - 1],
                            rhs=ens[dt_i - 2],
                            start=(dt_i - 2 == 0),
                            stop=False,
                        )
                    en_sb = work.tile([P, width], BF16, tag="energy")
                    nc.scalar.activation(
                        out=en_sb,
                        in_=ps_e,
                        func=AFT.Tanh,
                        bias=decT_sb[:, dt_i, b : b + 1],
                    )
                    ens.append(en_sb)
                for dt_i in (DT - 2, DT - 1):
                    nc.tensor.matmul(
                        ps_s,
                        lhsT=vt_sb[:, dt_i : dt_i + 1],
                        rhs=ens[dt_i],
                        start=False,
                        stop=(dt_i == DT - 1),
                    )
                # psum -> scores row (with additive mask fold-in)
                nc.vector.tensor_add(
                    out=scores_sb[:, off * P : off * P + width],
                    in0=ps_s,
                    in1=mn_sb[:, off * P : off * P + width],
                )
                # previous batch's context hides under this batch's first chunk
                if ci == 0 and prev is not None:
                    emit_context(prev)
                    prev = None
                # natural copy for the context matmul (one contiguous load,
                # emitted after the first transposes so it doesn't block them)
                if ci == min(2, NCH - 1):
                    xn_sb = xn_pool.tile([P, ST, E], BF16, tag="xn")
                    nc.scalar.dma_start(
                        out=xn_sb,
                        in_=Xd.ap()[b].rearrange("(st p) e -> p st e", p=P),
                    )

            # softmax over s on partition 0 (in-place on scores_sb)
            negmax = sm_pool.tile([1, 1], F32, tag="negmax")
            nc.vector.tensor_reduce(
                out=negmax,
                in_=scores_sb,
                axis=mybir.AxisListType.X,
                op=mybir.AluOpType.max,
                negate=True,
            )
            ssum = sm_pool.tile([1, 1], F32, tag="ssum")
            nc.scalar.activation(
                out=scores_sb,
                in_=scores_sb,
                func=AFT.Exp,
                bias=negmax,
                accum_out=ssum,
            )
            rinv = sm_pool.tile([1, 1], F32, tag="rinv")
            nc.vector.reciprocal(out=rinv, in_=ssum)

            # w^T via DRAM bounce (unnormalized exp row — context scales by
            # rinv): flat write, strided cast read to [sp, st]
            wq_dram = dram_pool.tile([S], F32, tag="wq")
            nc.gpsimd.dma_start(out=wq_dram, in_=scores_sb)
            wt_sb = sm_pool.tile([P, ST], BF16, tag="wt")
            nc.gpsimd.dma_start(
                out=wt_sb, in_=wq_dram[:].rearrange("(st sp) -> sp st", sp=P)
            )
            # normalize -> weights output row
            nc.vector.tensor_scalar_mul(scores_sb, scores_sb, rinv)
            nc.gpsimd.dma_start(out=WTSd.ap()[b : b + 1, :], in_=scores_sb)
            prev = (b, xn_sb, wt_sb, rinv)

        emit_context(prev)

    nc.compile()
    return nc


def host_inputs(decoder_hidden, encoder_outputs, mask, W_enc, W_dec, v,
                n_cores=N_CORES):
    """Shard + prep host-side numpy inputs; returns per-core input maps."""
    nb, Dd = decoder_hidden.shape
    bl = nb // n_cores
    Ee = W_enc.shape[0]
    ET, DT = Ee // 128, Dd // 128
    maskneg = np.where(mask, np.float32(-1e30), np.float32(0.0)).astype(np.float32)
    xb = encoder_outputs.astype(ml_dtypes.bfloat16)
    # W_enc in dt-major SBUF layout [p, dt, et, c]
    wenc_b = np.ascontiguousarray(
        W_enc.astype(ml_dtypes.bfloat16)
        .reshape(ET, 128, DT, 128)
        .transpose(1, 2, 0, 3)
    )
    # W_dec in strip-major SBUF layout [dt, p, et, c]
    wdec_b = np.ascontiguousarray(
        W_dec.astype(ml_dtypes.bfloat16)
        .reshape(ET, 128, DT, 128)
        .transpose(1, 2, 0, 3)
    )
    vt = np.ascontiguousarray(v.astype(ml_dtypes.bfloat16).reshape(DT, 128).T)
    hT = np.ascontiguousarray(decoder_hidden.T).astype(ml_dtypes.bfloat16)
    in_maps = []
    for c in range(n_cores):
        sl = slice(c * bl, (c + 1) * bl)
        in_maps.append(
            {
                "x": np.ascontiguousarray(xb[sl]),
                "ht": np.ascontiguousarray(hT[:, sl]),
                "maskneg": np.ascontiguousarray(maskneg[sl]),
                "wenc": wenc_b,
                "wdec": wdec_b,
                "vt": vt,
            }
        )
    return in_maps


_CACHE = {}


def _get_program():
    if "nc" not in _CACHE:
        _CACHE["nc"] = build_program(BL=BL, S=S, E=E, D=D)
    return _CACHE["nc"]


def run(inputs, trace=False):
    """inputs: dict as from setup_inputs(); returns (results, context, weights)."""
    nc = _get_program()
    in_maps = host_inputs(
        np.asarray(inputs["decoder_hidden"], dtype=np.float32),
        np.asarray(inputs["encoder_outputs"], dtype=np.float32),
        np.asarray(inputs["mask"]),
        np.asarray(inputs["W_enc"], dtype=np.float32),
        np.asarray(inputs["W_dec"], dtype=np.float32),
        np.asarray(inputs["v"], dtype=np.float32),
    )
    res = run_bass_kernel_spmd(nc, in_maps, list(range(N_CORES)), trace=trace)
    context = np.concatenate(
        [np.asarray(r["context"], dtype=np.float32) for r in res.results], axis=0
    )
    weights = np.concatenate(
        [np.asarray(r["weights"], dtype=np.float32) for r in res.results], axis=0
    )
    return res, context, weights


def kernel(decoder_hidden, encoder_outputs, mask, W_enc, W_dec, v):
    _, context, weights = run(
        {
            "decoder_hidden": decoder_hidden,
            "encoder_outputs": encoder_outputs,
            "mask": mask,
            "W_enc": W_enc,
            "W_dec": W_dec,
            "v": v,
        }
    )
    return context, weights
